# revision 15
# baseline (speedup 1.0000x reference)
"""Trainium2 Bass kernel for nn_Attention_11046655885816.

Full inputs in, full output out. Internally: 8 NeuronCores, each core
handles (one batch, a slice of heads). Projections + attention run
on-device in fp16/bf16 with fp32 PSUM accumulation; the softmax
denominator is produced by appending a key-mask column to the value
matrix, and the final divide + head assembly happens on the host.

Key layout choices (per core):
  qT, kT   : [64*NH partitions (head-major), L]  (fp16)  -> scores need no
             transposes anywhere: S^T tile = kT_tile.T @ qT.
  v_aug    : [Lk partitions, NH*(64+1)]  (bf16) -- per head 64 value cols
             plus one kmask column; AV matmul then yields numerator and
             denominator in one accumulation group.
  exp      : ScalarE reads score PSUM quads [128, 3*512] directly and
             writes bf16 T tiles to SBUF.
No max-subtraction is needed: scores are O(+-60) and exp stays inside
fp32/bf16 range; masked keys contribute exactly zero via the zeroed
v_aug rows (V_seq columns are zeroed host-side past V_len).
"""

import math
import os
import numpy as np
import ml_dtypes

B, L, D = 4, 2048, 1024
H, DH = 16, 64

_nc_cache = {}
LAST_EXEC_NS = None
LAST_SPMD_WALL_NS = None
LAST_RESULT = None


def _build(cfg):
    """Build + compile the per-core Bass program for a launch config.

    cfg keys: NH (heads/core, even), LQ, LK (multiples of 128).
    """
    import concourse.bass as bass
    import concourse.mybir as mybir
    import concourse.tile as tile
    from concourse import bacc

    NH = cfg["NH"]
    LQ = cfg["LQ"]
    LK = cfg["LK"]
    assert NH % 2 == 0 and LQ % 128 == 0 and LK % 128 == 0
    EH = NH * DH                 # E columns on this core
    NEB = EH // 128              # E blocks == head pairs
    ND = D // 128                # contraction tiles for projections
    NTK = LK // 128              # lk tiles
    NLQB = LQ // 128             # lq blocks
    VW = DH + 1                  # value cols + mask col per head

    # lk quads: up to 12 tiles of [128, 128] packed into one [128, 1536]
    # 3-bank PSUM region (scores for one 128-wide lq block)
    quads = []
    t = 0
    while t < NTK:
        n = min(12, NTK - t)
        quads.append((t, n))
        t += n

    fp16 = mybir.dt.float16
    bf16 = mybir.dt.bfloat16
    f32 = mybir.dt.float32

    # Per-head-pair arena strides padded to 8 KiB: base_partition=64
    # matmul operands at free-offsets that are odd multiples of 4 KiB
    # returned corrupted scores on HW; 8 KiB-aligned slices are clean.
    LKS = ((LK * 2 + 8191) // 8192) * 4096
    LQS = ((LQ * 2 + 8191) // 8192) * 4096

    nc = bacc.Bacc(
        "TRN2", target_bir_lowering=False, debug=False, num_devices=8
    )

    xq = nc.dram_tensor("xq", [D, LQ], fp16, kind="ExternalInput").ap()
    xk = nc.dram_tensor("xk", [D, LK], fp16, kind="ExternalInput").ap()
    xv = nc.dram_tensor("xv", [D, LK], fp16, kind="ExternalInput").ap()
    wq = nc.dram_tensor("wq", [D, EH], fp16, kind="ExternalInput").ap()
    wk = nc.dram_tensor("wk", [D, EH], fp16, kind="ExternalInput").ap()
    wv = nc.dram_tensor("wv", [D, EH], fp16, kind="ExternalInput").ap()
    km = nc.dram_tensor("km", [128, NTK * NH], bf16, kind="ExternalInput").ap()
    outp = nc.dram_tensor("outp", [LQ, NH * VW], f32, kind="ExternalOutput").ap()

    with tile.TileContext(nc, trace_sim=False) as tc:
        with (
            tc.tile_pool(name="xc", bufs=2) as xc_pool,
            tc.tile_pool(name="win", bufs=1) as win_pool,
            tc.tile_pool(name="proj", bufs=1) as proj_pool,
            tc.tile_pool(name="tsb", bufs=4) as t_pool,
            tc.tile_pool(name="osb", bufs=4) as o_pool,
            tc.tile_pool(name="ps", bufs=2, space="PSUM") as pp_pool,
            tc.tile_pool(name="pav", bufs=2, space="PSUM") as pav_pool,
        ):
            # ---- persistent SBUF arenas ----
            wq_sb = win_pool.tile([128, ND * EH], fp16, tag="wq")
            wk_sb = win_pool.tile([128, ND * EH], fp16, tag="wk")
            wv_sb = win_pool.tile([128, ND * EH], fp16, tag="wv")
            qt_sb = proj_pool.tile([128, NEB * LQS], fp16, tag="qt")
            kt_sb = proj_pool.tile([128, NEB * LKS], fp16, tag="kt")
            v_sb = proj_pool.tile([128, NTK * NH * VW], bf16, tag="v")

            # ---- weight + kmask DMAs ----
            for dt in range(ND):
                nc.sync.dma_start(
                    wv_sb[:, dt * EH : (dt + 1) * EH],
                    wv[dt * 128 : (dt + 1) * 128, :],
                )
                nc.sync.dma_start(
                    wk_sb[:, dt * EH : (dt + 1) * EH],
                    wk[dt * 128 : (dt + 1) * 128, :],
                )
                nc.sync.dma_start(
                    wq_sb[:, dt * EH : (dt + 1) * EH],
                    wq[dt * 128 : (dt + 1) * 128, :],
                )
            v4 = v_sb[:].rearrange("p (t h c) -> p t h c", t=NTK, h=NH, c=VW)
            nc.sync.dma_start(
                v4[:, :, :, DH],
                km.rearrange("p (t h) -> p t h", h=NH),
            )

            def stream_x(src):
                """DMA one 512-wide L-chunk of all D-tiles into a fresh tile."""
                def get(lc, w):
                    xc = xc_pool.tile([128, ND * 512], fp16, tag="xc")
                    for dt in range(ND):
                        nc.sync.dma_start(
                            xc[:, dt * 512 : dt * 512 + w],
                            src[dt * 128 : (dt + 1) * 128, lc : lc + w],
                        )
                    return xc
                return get

            get_xv = stream_x(xv)
            get_xk = stream_x(xk)
            get_xq = stream_x(xq)

            # ---- projections ----
            # v: normal layout [lk, E]; stationary = xv tile, moving = wv
            for lc in range(0, LK, 512):
                w = min(512, LK - lc)
                xcv = get_xv(lc, w)
                for t4 in range((w + 127) // 128):
                    t = lc // 128 + t4
                    ps = pp_pool.tile([128, 1536], f32, tag="sq")
                    for dt in range(ND):
                        nc.tensor.matmul(
                            ps[:, :EH],
                            lhsT=xcv[:, dt * 512 + t4 * 128 : dt * 512 + (t4 + 1) * 128],
                            rhs=wv_sb[:, dt * EH : (dt + 1) * EH],
                            start=(dt == 0),
                            stop=(dt == ND - 1),
                        )
                    nc.vector.tensor_copy(
                        v4[:, t, :, 0:DH],
                        ps[:, :EH].rearrange("p (h e) -> p h e", h=NH, e=DH),
                    )

            # k, q: transposed layout [E, L]; stationary = W block, moving = xT
            for lc in range(0, LK, 512):
                w = min(512, LK - lc)
                xck = get_xk(lc, w)
                for eb in range(NEB):
                    ps = pp_pool.tile([128, 1536], f32, tag="sq")
                    for dt in range(ND):
                        nc.tensor.matmul(
                            ps[:, :w],
                            lhsT=wk_sb[:, dt * EH + eb * 128 : dt * EH + (eb + 1) * 128],
                            rhs=xck[:, dt * 512 : dt * 512 + w],
                            start=(dt == 0),
                            stop=(dt == ND - 1),
                        )
                    nc.vector.tensor_copy(
                        kt_sb[:, eb * LKS + lc : eb * LKS + lc + w], ps[:, :w]
                    )
            for lc in range(0, LQ, 512):
                w = min(512, LQ - lc)
                xcq = get_xq(lc, w)
                for eb in range(NEB):
                    ps = pp_pool.tile([128, 1536], f32, tag="sq")
                    for dt in range(ND):
                        nc.tensor.matmul(
                            ps[:, :w],
                            lhsT=wq_sb[:, dt * EH + eb * 128 : dt * EH + (eb + 1) * 128],
                            rhs=xcq[:, dt * 512 : dt * 512 + w],
                            start=(dt == 0),
                            stop=(dt == ND - 1),
                        )
                    nc.vector.tensor_copy(
                        qt_sb[:, eb * LQS + lc : eb * LQS + lc + w], ps[:, :w]
                    )

            # ---- attention ----
            for hp in range(NEB):
                hA, hB = 2 * hp, 2 * hp + 1
                for lb in range(NLQB):
                    lqs = lb * 128
                    pavA = pav_pool.tile([128, VW], f32, tag="av")
                    pavB = pav_pool.tile([128, VW], f32, tag="av")
                    for qi, (t0, tn) in enumerate(quads):
                        psA = pp_pool.tile([128, 1536], f32, tag="sq")
                        psB = pp_pool.tile([128, 1536], f32, tag="sq")
                        for j in range(tn):
                            tt = t0 + j
                            nc.tensor.matmul(
                                psA[:, j * 128 : (j + 1) * 128],
                                lhsT=kt_sb[0:64, hp * LKS + tt * 128 : hp * LKS + (tt + 1) * 128],
                                rhs=qt_sb[0:64, hp * LQS + lqs : hp * LQS + lqs + 128],
                                start=True,
                                stop=True,
                            )
                            nc.tensor.matmul(
                                psB[:, j * 128 : (j + 1) * 128],
                                lhsT=kt_sb[64:128, hp * LKS + tt * 128 : hp * LKS + (tt + 1) * 128],
                                rhs=qt_sb[64:128, hp * LQS + lqs : hp * LQS + lqs + 128],
                                start=True,
                                stop=True,
                            )
                        w_all = tn * 128
                        tA = t_pool.tile([128, 1536], bf16, tag="t")
                        tB = t_pool.tile([128, 1536], bf16, tag="t")
                        nc.scalar.activation(
                            tA[:, :w_all], psA[:, :w_all],
                            mybir.ActivationFunctionType.Exp,
                        )
                        nc.scalar.activation(
                            tB[:, :w_all], psB[:, :w_all],
                            mybir.ActivationFunctionType.Exp,
                        )
                        first = qi == 0
                        last = qi == len(quads) - 1
                        for j in range(tn):
                            tt = t0 + j
                            nc.tensor.matmul(
                                pavA[:, 0:VW],
                                lhsT=tA[:, j * 128 : (j + 1) * 128],
                                rhs=v4[:, tt, hA, :],
                                start=first and j == 0,
                                stop=last and j == tn - 1,
                            )
                            nc.tensor.matmul(
                                pavB[:, 0:VW],
                                lhsT=tB[:, j * 128 : (j + 1) * 128],
                                rhs=v4[:, tt, hB, :],
                                start=first and j == 0,
                                stop=last and j == tn - 1,
                            )
                    # epilogue: PSUM -> SBUF -> DRAM (divide happens on host)
                    oA = o_pool.tile([128, VW], f32, tag="o")
                    oB = o_pool.tile([128, VW], f32, tag="o")
                    nc.vector.tensor_copy(oA[:, :], pavA[:, :])
                    nc.vector.tensor_copy(oB[:, :], pavB[:, :])
                    nc.sync.dma_start(
                        outp[lqs : lqs + 128, hA * VW : (hA + 1) * VW], oA[:, :]
                    )
                    nc.sync.dma_start(
                        outp[lqs : lqs + 128, hB * VW : (hB + 1) * VW], oB[:, :]
                    )

    nc.compile()
    return nc


def _get_nc(cfg):
    key = tuple(sorted(cfg.items()))
    if key not in _nc_cache:
        _nc_cache[key] = _build(cfg)
    return _nc_cache[key]


def _prep_core_inputs(Xq, Xk, Xv, Wq, Wk, Wv, vlen, cfg):
    """Host-side slicing/transposition/masking for one core.

    Xq/Xk/Xv: [L, D] fp32 for this batch; W*: [D, EH] slices for this
    core's heads; vlen: effective V_len (0 means "no mask").
    """
    NH, LQ, LK = cfg["NH"], cfg["LQ"], cfg["LK"]
    f16 = np.float16
    bf16 = ml_dtypes.bfloat16

    NTK = LK // 128
    xq = np.zeros((D, LQ), f16)
    xq[:, : min(LQ, L)] = Xq[: min(LQ, L)].T.astype(f16)
    xk = np.zeros((D, LK), f16)
    xv = np.zeros((D, LK), f16)
    n = min(LK, L) if vlen == 0 else min(LK, vlen)
    xk[:, :n] = Xk[:n].T.astype(f16)
    xv[:, :n] = Xv[:n].T.astype(f16)
    kmask = (np.arange(LK) < n).astype(np.float32)
    # device layout [128, NTK*NH]: km[p, t*NH + h] = kmask[t*128 + p]
    kmv = np.repeat(
        kmask.reshape(NTK, 128).T[:, :, None], NH, axis=2
    ).reshape(128, NTK * NH)
    return {
        "xq": xq,
        "xk": xk,
        "xv": xv,
        "wq": np.ascontiguousarray(Wq, dtype=f16),
        "wk": np.ascontiguousarray(Wk, dtype=f16),
        "wv": np.ascontiguousarray(Wv, dtype=f16),
        "km": kmv.astype(bf16),
    }


def kernel(Q_seq, K_seq, V_seq, Q_len, V_len, WQ, WK, WV):
    from concourse.bass_utils import run_bass_kernel_spmd

    Q_seq = np.asarray(Q_seq, np.float32)
    K_seq = np.asarray(K_seq, np.float32)
    V_seq = np.asarray(V_seq, np.float32)
    WQ = np.asarray(WQ, np.float32)
    WK = np.asarray(WK, np.float32)
    WV = np.asarray(WV, np.float32)
    q_len = np.asarray(Q_len).reshape(-1).astype(np.int64)
    v_len = np.asarray(V_len).reshape(-1).astype(np.int64)

    cfg = {"NH": 8, "LQ": 2048, "LK": 2048}
    NH, LQ, LK = cfg["NH"], cfg["LQ"], cfg["LK"]
    VW = DH + 1
    nc = _get_nc(cfg)

    in_maps = []
    core_meta = []
    for b in range(B):
        for hg in range(2):
            e0, e1 = hg * NH * DH, (hg + 1) * NH * DH
            m = _prep_core_inputs(
                Q_seq[b], K_seq[b], V_seq[b],
                WQ[:, e0:e1], WK[:, e0:e1], WV[:, e0:e1],
                int(v_len[b]), cfg,
            )
            in_maps.append(m)
            core_meta.append((b, hg))

    import time as _time

    trace = os.environ.get("NN_ATT_TRACE") == "1"
    t_spmd = _time.time()
    try:
        res = run_bass_kernel_spmd(
            nc, in_maps, core_ids=list(range(8)), trace=trace,
            **({"trace_cores": list(range(8))} if trace else {}),
        )
    except Exception:
        if not trace:
            raise
        res = run_bass_kernel_spmd(nc, in_maps, core_ids=list(range(8)))
    global LAST_EXEC_NS, LAST_RESULT, LAST_SPMD_WALL_NS
    LAST_SPMD_WALL_NS = int((_time.time() - t_spmd) * 1e9)
    LAST_RESULT = res
    if res.exec_time_ns:
        LAST_EXEC_NS = int(res.exec_time_ns)

    out = np.zeros((B, L, H * DH), np.float32)
    for c, (b, hg) in enumerate(core_meta):
        arr = res.results[c]["outp"]  # [LQ, NH*VW]
        nq = min(int(q_len[b]), LQ, L)
        if nq <= 0:
            continue
        a = arr[:nq].reshape(nq, NH, VW)
        num = a[:, :, :DH]
        den = a[:, :, DH:DH + 1]
        o = num / den
        out[b, :nq, hg * NH * DH : (hg + 1) * NH * DH] = o.reshape(nq, NH * DH)
    return out
